# revision 72
# baseline (speedup 1.0000x reference)
"""Gated dual-score (semantic+geometric/RoPE) causal attention layer on 8 TRN2 cores.

Sharding: data-parallel over batch (2) x tensor-parallel over heads (16 -> 4/core).
Core i: batch b = i // 4, heads hg = i % 4 -> heads [4*hg, 4*hg+4).
Each core computes a partial y (its heads' contribution, its batch); the host
sums the 4 partials per batch (the "all-reduce" of the row-sharded out-proj).

On-device layout: all projections consume xT (d on partitions, t free) and
produce qT/kT in (d, t) layout. Scores are computed transposed (s on
partitions, t free) so P@V consumes the exp tile directly with V in natural
(t, dv) layout. Key structure choices (all aimed at keeping the PE array,
the bottleneck at ~88% busy, free of non-matmul work):
  - Projections are stacked per head as [q_sem|k_sem] and [k_geo|q_geo] so
    RoPE/gating DVE ops run on full 128-partition tiles (q and k together).
  - The causal mask is folded into the score matmul as a second accumulated
    matmul adding a -1e9 upper-triangular constant (no DVE in exp->PV path).
  - The softmax denominator is accumulated on DVE (bf16 adds of exp tiles)
    and turned into a broadcast row-sum by ONE ones(128x128) matmul per
    chunk, replacing per-tile ones-vector matmuls on the PE.
  - x / weights stream via per-(chunk,k) contiguous DMAs ordered by first
    use on the sync ring; wv/wo/y-stores use the scalar (ACT) ring.
Softmax skips max-subtraction (scores are O(5) by construction).
Compute dtype bf16 (fp32 matmul costs 4x cycles on TRN2), fp32 accumulation.
"""

import sys
from contextlib import ExitStack

import numpy as np

sys.path.insert(0, "/opt/trn_rl_repo")

import ml_dtypes  # noqa: E402

import concourse.bass as bass  # noqa: E402
from concourse import bacc  # noqa: E402
import concourse.mybir as mybir  # noqa: E402
import concourse.tile as tile  # noqa: E402
from concourse.bass_utils import run_bass_kernel_spmd  # noqa: E402

B, T, D, H = 2, 2048, 2048, 16
SEM_HD = GEO_HD = 64
V_HD = 128
HL = 4  # heads per core
CL = HL * V_HD  # local v-dim (512)
ROPE_BASE = 10000.0
NEG_INF = -1e9

KT = D // 128  # 16 k-tiles over the contraction dim
TT = T // 128  # 16 token tiles of 128
TC = T // 512  # 4 token chunks of 512
BF = mybir.dt.bfloat16
F32 = mybir.dt.float32
NPBF = ml_dtypes.bfloat16

_CACHED_NC = None


def _build_nc():
    nc = bacc.Bacc()

    # cpack columns: crep | srep | ident | mbias | selc (rows 0:8)
    CP = 2 * T + 128 + 128 + HL * 128
    xt_d = nc.declare_dram_parameter("xt", [128, TC, KT, 512], BF, isOutput=False)
    wqk_d = nc.declare_dram_parameter("wqk", [HL, 128, 2, KT, 128], BF, isOutput=False)
    wv_d = nc.declare_dram_parameter("wv", [128, KT, CL], BF, isOutput=False)
    wo_d = nc.declare_dram_parameter("wo", [HL, 128, D], BF, isOutput=False)
    wg_d = nc.declare_dram_parameter("wg", [128, KT, 2 * HL], BF, isOutput=False)
    cpack_d = nc.declare_dram_parameter("cpack", [128, CP], BF, isOutput=False)
    gpack_d = nc.declare_dram_parameter("gpack", [2 * HL, 3], F32, isOutput=False)
    y_d = nc.declare_dram_parameter("y", [T, D], BF, isOutput=True)

    with tile.TileContext(nc) as tc, ExitStack() as ctx:
        singles = ctx.enter_context(tc.tile_pool(name="singles", bufs=1))
        xpool = ctx.enter_context(tc.tile_pool(name="xpool", bufs=1))
        vpool = ctx.enter_context(tc.tile_pool(name="vpool", bufs=1))
        wqk_pool = ctx.enter_context(tc.tile_pool(name="wqk", bufs=2))
        qk_pool = ctx.enter_context(tc.tile_pool(name="qk", bufs=2))
        ot_pool = ctx.enter_context(tc.tile_pool(name="ot", bufs=1))
        wo_pool = ctx.enter_context(tc.tile_pool(name="wo", bufs=1))
        p_pool = ctx.enter_context(tc.tile_pool(name="pp", bufs=8))
        sc_pool = ctx.enter_context(tc.tile_pool(name="scratch", bufs=2))
        y_pool = ctx.enter_context(tc.tile_pool(name="ysb", bufs=2))

        ps_big = ctx.enter_context(tc.tile_pool(name="ps_big", bufs=3, space="PSUM"))
        ps_s = ctx.enter_context(tc.tile_pool(name="ps_s", bufs=2, space="PSUM"))
        ps_o = ctx.enter_context(tc.tile_pool(name="ps_o", bufs=2, space="PSUM"))
        ps_r = ctx.enter_context(tc.tile_pool(name="ps_r", bufs=1, space="PSUM"))

        # ---- sync ring: wg/gpack first (the first matmul needs wg), then xT
        # chunk by chunk; per-head q/k weights follow chunk 0.
        wg = singles.tile([128, KT, 2 * HL], BF)
        nc.sync.dma_start(out=wg, in_=wg_d[:])
        ones128 = singles.tile([128, 128], BF)
        nc.vector.memset(ones128, 1.0)



        xt = xpool.tile([128, TC, KT, 512], BF)
        nc.sync.dma_start(out=xt[:, 0, 0:6], in_=xt_d[:, 0, 0:6])
        nc.sync.dma_start(out=xt[:, 0, 6:KT], in_=xt_d[:, 0, 6:KT])

        # gate sigmoid params: first consumer is ~12us after the first matmul
        gpack = singles.tile([2 * HL, 3], F32)
        nc.sync.dma_start(out=gpack, in_=gpack_d[:])
        glog = gpack[:, 0:1]
        gsv = gpack[:, 1:2]  # [1/8 x4; -1/8 x4]
        gbv = gpack[:, 2:3]  # [0 x4; 1/8 x4]

        wqk0 = wqk_pool.tile([128, 2, KT, 128], BF, tag="wqk", name="wqk0")
        nc.sync.dma_start(out=wqk0, in_=wqk_d[0])

        # packed constants (rope tables et al): needed once head-0's first
        # projection chain completes
        cpack = singles.tile([128, CP], BF)
        nc.sync.dma_start(out=cpack, in_=cpack_d[:])
        crep = cpack[:, 0:T]
        srep = cpack[:, T : 2 * T]
        ident = cpack[:, 2 * T : 2 * T + 128]
        mbias = cpack[:, 2 * T + 128 : 2 * T + 256]
        selc = cpack[0 : 2 * HL, 2 * T + 256 : 2 * T + 256 + HL * 128]

        for j in range(1, TC):
            nc.sync.dma_start(out=xt[:, j], in_=xt_d[:, j])

        wv_pool = ctx.enter_context(tc.tile_pool(name="wvpool", bufs=1))
        wv = wv_pool.tile([128, KT, CL], BF)
        nc.sync.dma_start(out=wv, in_=wv_d[:])
        wo_sb = [
            wo_pool.tile([128, D], BF, tag=f"wo{h}", name=f"wo{h}") for h in range(HL)
        ]

        # ---- helpers ----
        gcomb = singles.tile([2 * HL, T], BF)  # rows 0:4 g/8, rows 4:8 (1-g)/8

        def gate_chunk(j):
            tsl = slice(512 * j, 512 * (j + 1))
            pg = ps_s.tile([2 * HL, 512], F32, tag="ps", name="pg")
            for k in range(KT):
                nc.tensor.matmul(
                    pg, wg[:, k, :], xt[:, j, k, :], start=(k == 0), stop=(k == KT - 1)
                )
            gsig = sc_pool.tile([2 * HL, 512], F32, tag="gsig", bufs=1)
            nc.scalar.activation(
                gsig, pg, mybir.ActivationFunctionType.Sigmoid, bias=glog
            )
            nc.scalar.activation(
                gcomb[:, tsl],
                gsig,
                mybir.ActivationFunctionType.Identity,
                scale=gsv,
                bias=gbv,
            )

        def proj_chunk(h, j, wqk_sb, qstk, kstk):
            wsem_sb, wgeo_sb = wqk_sb[:, 0], wqk_sb[:, 1]
            """QK projection for head h, token chunk j.

            p_sem rows: [q_sem(0:64) | k_sem(64:128)]
            p_geo rows: [k_geo(0:64) | q_geo(64:128)]
            gbs  rows: [g/8   (0:64) | (1-g)/8 (64:128)]  (q-side scales)
            """
            tsl = slice(512 * j, 512 * (j + 1))
            p_sem = ps_big.tile([128, 512], F32, tag="big", name="p_sem")
            p_geo = ps_big.tile([128, 512], F32, tag="big", name="p_geo")
            for k in range(KT):
                nc.tensor.matmul(
                    p_sem, wsem_sb[:, k, :], xt[:, j, k, :],
                    start=(k == 0), stop=(k == KT - 1),
                )
            for k in range(KT):
                nc.tensor.matmul(
                    p_geo, wgeo_sb[:, k, :], xt[:, j, k, :],
                    start=(k == 0), stop=(k == KT - 1),
                )
            gbb = ps_big.tile([128, 512], F32, tag="big", name="gbb")
            nc.tensor.matmul(
                gbb, selc[:, 128 * h : 128 * (h + 1)], gcomb[:, tsl],
                start=True, stop=True,
            )
            gbs = sc_pool.tile([128, 512], BF, tag="gbs", bufs=2)
            nc.scalar.copy(gbs, gbb)

            # RoPE on the stacked geo tile (all 128 partitions per op)
            m1 = sc_pool.tile([128, 512], BF, tag="m1", bufs=2)
            m2 = sc_pool.tile([128, 512], BF, tag="m2", bufs=2)
            sw = sc_pool.tile([128, 512], BF, tag="sw", bufs=2)
            nc.vector.tensor_mul(m1, p_geo, crep[:, tsl])
            nc.vector.tensor_mul(m2, p_geo, srep[:, tsl])
            for blk in range(4):  # swap 32-row halves within each 64
                d0 = 64 * (blk // 2) + 32 * (blk % 2)
                s0 = 64 * (blk // 2) + 32 * (1 - blk % 2)
                # DVE: bf16 SBUF copies are ~287ns here vs ~716ns on ACT,
                # and ACT-queued copies head-of-line block the attention exps
                nc.vector.tensor_copy(sw[d0 : d0 + 32, :], m2[s0 : s0 + 32, :])
            nc.vector.tensor_add(m1, m1, sw)  # m1 = rotated [k_geo | q_geo]

            # q side gets the gate scales folded in; k side is passthrough
            nc.vector.tensor_mul(qstk[0:64, tsl], p_sem[0:64, :], gbs[0:64, :])
            nc.vector.tensor_mul(qstk[64:128, tsl], m1[64:128, :], gbs[64:128, :])
            nc.vector.tensor_copy(kstk[0:64, tsl], p_sem[64:128, :])
            nc.vector.tensor_copy(kstk[64:128, tsl], m1[0:64, :])

        # ---- gate + head-0 projection, chunk by chunk (tracks DMA arrival) ----
        # ---- V projection helper, natural (t, dv) layout ----
        v_sb = vpool.tile([128, TT, CL], BF)

        def v_tile(i):
            pv = ps_big.tile([128, CL], F32, tag="big", name="pv")
            for k in range(KT):
                nc.tensor.matmul(
                    pv,
                    xt[:, i // 4, k, 128 * (i % 4) : 128 * (i % 4 + 1)],
                    wv[:, k, :],
                    start=(k == 0),
                    stop=(k == KT - 1),
                )
            if i % 2 == 0:
                nc.scalar.copy(v_sb[:, i, :], pv)
            else:
                nc.vector.tensor_copy(v_sb[:, i, :], pv)

        qstk0 = qk_pool.tile([128, T], BF, tag="qstk", name="qstk0")
        kstk0 = qk_pool.tile([128, T], BF, tag="kstk", name="kstk0")
        for j in range(TC):
            gate_chunk(j)
            proj_chunk(0, j, wqk0, qstk0, kstk0)
        for i in range(TT):
            v_tile(i)

        # ---- per-head: attention, then next head's projection ----
        outT = [
            ot_pool.tile([128, T], BF, tag=f"ot{h}", name=f"ot{h}") for h in range(HL)
        ]
        qstk, kstk = qstk0, kstk0
        for h in range(HL):
            for j in range(TC):
                tsl = slice(512 * j, 512 * (j + 1))
                po = ps_o.tile([128, 512], F32, tag="po")
                acc = sc_pool.tile([128, 512], BF, tag="acc", bufs=2)
                n_s = 4 * (j + 1)
                fold = []
                for s in range(n_s):
                    dj = s - 4 * j  # >=0 on diagonal tiles
                    c0 = 128 * dj if dj >= 0 else 0
                    ssl = slice(128 * s, 128 * (s + 1))
                    ps = ps_s.tile([128, 512], F32, tag="ps", name="ps")
                    nc.tensor.matmul(
                        ps[:, c0:512],
                        kstk[:, ssl],
                        qstk[:, 512 * j + c0 : 512 * (j + 1)],
                        start=True,
                        stop=(dj < 0),
                        skip_group_check=(dj >= 0),
                    )
                    if dj >= 0:
                        # causal mask: add -1e9 upper triangle to the diag block
                        nc.tensor.matmul(
                            ps[:, c0 : c0 + 128],
                            ident,
                            mbias,
                            start=False,
                            stop=True,
                            skip_group_check=True,
                        )
                    pt = p_pool.tile([128, 512], BF, tag="pt", name="pt")
                    nc.scalar.activation(
                        pt[:, c0:512], ps[:, c0:512], mybir.ActivationFunctionType.Exp
                    )
                    if s == 0:
                        nc.vector.tensor_copy(acc, pt)
                    elif dj < 3:
                        nc.vector.tensor_add(
                            acc[:, c0:512], acc[:, c0:512], pt[:, c0:512]
                        )
                    else:
                        fold.append((pt, c0))
                    nc.tensor.matmul(
                        po[:, c0:512],
                        v_sb[:, s, 128 * h : 128 * (h + 1)],
                        pt[:, c0:512],
                        start=(s == 0),
                        stop=(s == n_s - 1),
                    )
                # denominator: broadcast partition-sums of acc; the partial
                # diag tiles skip the DVE add chain and fold in as extra PE
                # accumulations so the chain never tails into the next chunk
                rbc = ps_r.tile([128, 512], F32, tag="rbc", name="rbc")
                nc.tensor.matmul(
                    rbc, ones128, acc, start=True, stop=False, skip_group_check=True
                )
                for fi, (fpt, fc0) in enumerate(fold):
                    nc.tensor.matmul(
                        rbc[:, fc0:512],
                        ones128,
                        fpt[:, fc0:512],
                        start=False,
                        stop=(fi == len(fold) - 1),
                        skip_group_check=True,
                    )
                rbs = sc_pool.tile([128, 512], F32, tag="rbs", bufs=2)
                nc.vector.reciprocal_approx_fast(out=rbs, in_=rbc)
                nc.vector.tensor_mul(outT[h][:, tsl], po, rbs)

            if h + 1 < HL:
                wqk_sb = wqk_pool.tile([128, 2, KT, 128], BF, tag="wqk")
                nc.sync.dma_start(out=wqk_sb, in_=wqk_d[h + 1])
                if h == 2:  # out-proj weights: loads due ~250us, issue late
                    for hh in range(HL):
                        nc.sync.dma_start(out=wo_sb[hh], in_=wo_d[hh])
                qstk = qk_pool.tile([128, T], BF, tag="qstk")
                kstk = qk_pool.tile([128, T], BF, tag="kstk")
                for j in range(TC):
                    proj_chunk(h + 1, j, wqk_sb, qstk, kstk)

        # ---- out-projection: y[t, e] = sum_h outT_h^T @ wo_h ----
        for i in range(TT):
            ysb = y_pool.tile([128, D], BF, tag="ysb")
            for ec in range(D // 512):
                py = ps_big.tile([128, 512], F32, tag="big", name="py")
                for h in range(HL):
                    nc.tensor.matmul(
                        py,
                        outT[h][:, 128 * i : 128 * (i + 1)],
                        wo_sb[h][:, 512 * ec : 512 * (ec + 1)],
                        start=(h == 0),
                        stop=(h == HL - 1),
                    )
                if ec % 2 == 0:
                    nc.scalar.copy(ysb[:, 512 * ec : 512 * (ec + 1)], py)
                else:
                    nc.vector.tensor_copy(ysb[:, 512 * ec : 512 * (ec + 1)], py)
                if i >= TT - 2:
                    # stream the final tiles' stores so the end-of-kernel
                    # barrier isn't gated on one large trailing transfer
                    nc.scalar.dma_start(
                        out=y_d[128 * i : 128 * (i + 1), 512 * ec : 512 * (ec + 1)],
                        in_=ysb[:, 512 * ec : 512 * (ec + 1)],
                    )
            if i < TT - 2:
                nc.scalar.dma_start(out=y_d[128 * i : 128 * (i + 1), :], in_=ysb)

    nc.finalize()
    return nc


def _host_prep(x, w_q_sem, w_k_sem, w_q_geo, w_k_geo, w_v, w_out, gate_logit, gate_w):
    """Build the 8 per-core input maps (all numpy, bf16 where matmul-bound)."""
    half = GEO_HD // 2  # 32
    inv_freq = 1.0 / (ROPE_BASE ** (np.arange(half, dtype=np.float64) / half))
    pos = np.arange(T, dtype=np.float64)
    ang = pos[None, :] * inv_freq[:, None]  # (32, T)
    cos, sin = np.cos(ang), np.sin(ang)
    crep = np.empty((128, T), dtype=NPBF)
    srep = np.empty((128, T), dtype=NPBF)
    for b0 in (0, 64):
        crep[b0 : b0 + 32] = cos
        crep[b0 + 32 : b0 + 64] = cos
        srep[b0 : b0 + 32] = sin  # sw[0:32]=m2[32:64] needs +sin here
        srep[b0 + 32 : b0 + 64] = -sin  # sw[32:64]=m2[0:32] needs -sin here
    # rot[0:32] = p[0:32]*cos - p[32:64]*sin = m1[0:32] + (p[32:64]*srep[32:64])
    # rot[32:64] = p[32:64]*cos + p[0:32]*sin = m1[32:64] + (p[0:32]*srep[0:32])
    # (sw swaps the 32-blocks, so srep rows carry the sign of the *destination*)

    p_i = np.arange(128)
    mbias = np.where(p_i[:, None] <= p_i[None, :], 0.0, NEG_INF).astype(NPBF)
    ident = np.eye(128, dtype=NPBF)
    selc = np.zeros((128, HL * 128), dtype=NPBF)
    for h in range(HL):
        selc[h, 128 * h : 128 * h + 64] = 1.0
        selc[HL + h, 128 * h + 64 : 128 * h + 128] = 1.0
    cpack = np.concatenate(
        [crep, srep, ident, mbias, selc], axis=1
    )  # (128, 2T+256+512)

    def stack_heads(wa, wb):
        # per-head (D, 128) = [wa_head | wb_head], as (128, KT, 128) lhsT tiles
        out = []
        for h in range(H):
            blk = np.concatenate(
                [wa[:, 64 * h : 64 * (h + 1)], wb[:, 64 * h : 64 * (h + 1)]], axis=1
            )
            out.append(
                np.ascontiguousarray(
                    blk.reshape(KT, 128, 128).transpose(1, 0, 2)
                ).astype(NPBF)
            )
        return out

    wsem_all = stack_heads(w_q_sem, w_k_sem)  # [q_sem | k_sem]
    wgeo_all = stack_heads(w_k_geo, w_q_geo)  # [k_geo | q_geo]
    wqk_all = [
        np.ascontiguousarray(np.stack([ws, wgg], axis=1))  # (128, 2, KT, 128)
        for ws, wgg in zip(wsem_all, wgeo_all)
    ]

    xt_by_b = [
        np.ascontiguousarray(
            x[b].T.reshape(KT, 128, TC, 512).transpose(1, 2, 0, 3)
        ).astype(NPBF)
        for b in range(B)
    ]  # (128, TC, KT, 512): [p, j, k, c] = xT[128k+p, 512j+c]

    in_maps = []
    for core in range(8):
        b, hg = core // 4, core % 4
        heads = range(4 * hg, 4 * hg + 4)
        wqk = np.stack([wqk_all[h] for h in heads])
        wv = np.ascontiguousarray(
            w_v[:, CL * hg : CL * (hg + 1)].reshape(KT, 128, CL).transpose(1, 0, 2)
        ).astype(NPBF)
        wo = w_out[CL * hg : CL * (hg + 1), :].reshape(HL, 128, D).astype(NPBF)
        gwl = gate_w[:, 4 * hg : 4 * hg + 4]  # (D, 4)
        gw2 = np.concatenate([gwl, gwl], axis=1)  # (D, 8) duplicated
        wg = np.ascontiguousarray(
            gw2.reshape(KT, 128, 2 * HL).transpose(1, 0, 2)
        ).astype(NPBF)
        gll = gate_logit[4 * hg : 4 * hg + 4]
        glog = np.concatenate([gll, gll]).astype(np.float32)
        gsv = np.array([0.125] * HL + [-0.125] * HL, dtype=np.float32)
        gbv = np.array([0.0] * HL + [0.125] * HL, dtype=np.float32)
        gpack = np.ascontiguousarray(np.stack([glog, gsv, gbv], axis=1))  # (8, 3)
        in_maps.append(
            {
                "xt": xt_by_b[b],
                "wqk": wqk,
                "wv": wv,
                "wo": np.ascontiguousarray(wo),
                "wg": wg,
                "cpack": cpack,
                "gpack": gpack,
            }
        )
    return in_maps


def _run(inputs, trace=False):
    global _CACHED_NC
    if _CACHED_NC is None:
        _CACHED_NC = _build_nc()
    in_maps = _host_prep(**{k: np.asarray(v) for k, v in inputs.items()})
    res = run_bass_kernel_spmd(
        _CACHED_NC, in_maps, core_ids=list(range(8)), trace=trace
    )
    y = np.zeros((B, T, D), dtype=np.float32)
    for core in range(8):
        y[core // 4] += res.results[core]["y"].astype(np.float32)
    return y, res


def kernel(**inputs) -> np.ndarray:
    y, _ = _run(inputs, trace=False)
    return y


# revision 73
# speedup vs baseline: 1.0077x; 1.0077x over previous
"""Gated dual-score (semantic+geometric/RoPE) causal attention layer on 8 TRN2 cores.

Sharding: data-parallel over batch (2) x tensor-parallel over heads (16 -> 4/core).
Core i: batch b = i // 4, heads hg = i % 4 -> heads [4*hg, 4*hg+4).
Each core computes a partial y (its heads' contribution, its batch); the host
sums the 4 partials per batch (the "all-reduce" of the row-sharded out-proj).

On-device layout: all projections consume xT (d on partitions, t free) and
produce qT/kT in (d, t) layout. Scores are computed transposed (s on
partitions, t free) so P@V consumes the exp tile directly with V in natural
(t, dv) layout. Key structure choices (all aimed at keeping the PE array,
the bottleneck at ~88% busy, free of non-matmul work):
  - Projections are stacked per head as [q_sem|k_sem] and [k_geo|q_geo] so
    RoPE/gating DVE ops run on full 128-partition tiles (q and k together).
  - The causal mask is folded into the score matmul as a second accumulated
    matmul adding a -1e9 upper-triangular constant (no DVE in exp->PV path).
  - The softmax denominator is accumulated on DVE (bf16 adds of exp tiles)
    and turned into a broadcast row-sum by ONE ones(128x128) matmul per
    chunk, replacing per-tile ones-vector matmuls on the PE.
  - x / weights stream via per-(chunk,k) contiguous DMAs ordered by first
    use on the sync ring; wv/wo/y-stores use the scalar (ACT) ring.
Softmax skips max-subtraction (scores are O(5) by construction).
Compute dtype bf16 (fp32 matmul costs 4x cycles on TRN2), fp32 accumulation.
"""

import sys
from contextlib import ExitStack

import numpy as np

sys.path.insert(0, "/opt/trn_rl_repo")

import ml_dtypes  # noqa: E402

import concourse.bass as bass  # noqa: E402
from concourse import bacc  # noqa: E402
import concourse.mybir as mybir  # noqa: E402
import concourse.tile as tile  # noqa: E402
from concourse.bass_utils import run_bass_kernel_spmd  # noqa: E402

B, T, D, H = 2, 2048, 2048, 16
SEM_HD = GEO_HD = 64
V_HD = 128
HL = 4  # heads per core
CL = HL * V_HD  # local v-dim (512)
ROPE_BASE = 10000.0
NEG_INF = -1e9

KT = D // 128  # 16 k-tiles over the contraction dim
TT = T // 128  # 16 token tiles of 128
TC = T // 512  # 4 token chunks of 512
BF = mybir.dt.bfloat16
F32 = mybir.dt.float32
NPBF = ml_dtypes.bfloat16

_CACHED_NC = None


def _build_nc():
    nc = bacc.Bacc()

    # cpack columns: crep | srep | ident | mbias | selc (rows 0:8)
    CP = 2 * T + 128 + 128 + HL * 128
    xt_d = nc.declare_dram_parameter("xt", [128, TC, KT, 512], BF, isOutput=False)
    wqk_d = nc.declare_dram_parameter("wqk", [HL, 128, 2, KT, 128], BF, isOutput=False)
    wv_d = nc.declare_dram_parameter("wv", [128, KT, CL], BF, isOutput=False)
    wo_d = nc.declare_dram_parameter("wo", [HL, 128, D], BF, isOutput=False)
    wg_d = nc.declare_dram_parameter("wg", [128, KT, 2 * HL], BF, isOutput=False)
    cpack_d = nc.declare_dram_parameter("cpack", [128, CP], BF, isOutput=False)
    gpack_d = nc.declare_dram_parameter("gpack", [2 * HL, 3], F32, isOutput=False)
    y_d = nc.declare_dram_parameter("y", [T, D], BF, isOutput=True)

    with tile.TileContext(nc) as tc, ExitStack() as ctx:
        singles = ctx.enter_context(tc.tile_pool(name="singles", bufs=1))
        xpool = ctx.enter_context(tc.tile_pool(name="xpool", bufs=1))
        vpool = ctx.enter_context(tc.tile_pool(name="vpool", bufs=1))
        wqk_pool = ctx.enter_context(tc.tile_pool(name="wqk", bufs=2))
        qk_pool = ctx.enter_context(tc.tile_pool(name="qk", bufs=2))
        ot_pool = ctx.enter_context(tc.tile_pool(name="ot", bufs=1))
        wo_pool = ctx.enter_context(tc.tile_pool(name="wo", bufs=1))
        p_pool = ctx.enter_context(tc.tile_pool(name="pp", bufs=8))
        sc_pool = ctx.enter_context(tc.tile_pool(name="scratch", bufs=2))
        y_pool = ctx.enter_context(tc.tile_pool(name="ysb", bufs=2))

        ps_big = ctx.enter_context(tc.tile_pool(name="ps_big", bufs=3, space="PSUM"))
        ps_s = ctx.enter_context(tc.tile_pool(name="ps_s", bufs=2, space="PSUM"))
        ps_o = ctx.enter_context(tc.tile_pool(name="ps_o", bufs=2, space="PSUM"))
        ps_r = ctx.enter_context(tc.tile_pool(name="ps_r", bufs=1, space="PSUM"))

        # ---- sync ring: wg/gpack first (the first matmul needs wg), then xT
        # chunk by chunk; per-head q/k weights follow chunk 0.
        wg = singles.tile([128, KT, 2 * HL], BF)
        nc.sync.dma_start(out=wg, in_=wg_d[:])
        ones128 = singles.tile([128, 128], BF)
        nc.vector.memset(ones128, 1.0)



        xt = xpool.tile([128, TC, KT, 512], BF)
        nc.sync.dma_start(out=xt[:, 0, 0:6], in_=xt_d[:, 0, 0:6])
        nc.sync.dma_start(out=xt[:, 0, 6:KT], in_=xt_d[:, 0, 6:KT])

        # gate sigmoid params: first consumer is ~12us after the first matmul
        gpack = singles.tile([2 * HL, 3], F32)
        nc.sync.dma_start(out=gpack, in_=gpack_d[:])
        glog = gpack[:, 0:1]
        gsv = gpack[:, 1:2]  # [1/8 x4; -1/8 x4]
        gbv = gpack[:, 2:3]  # [0 x4; 1/8 x4]

        wqk0 = wqk_pool.tile([128, 2, KT, 128], BF, tag="wqk", name="wqk0")
        nc.sync.dma_start(out=wqk0, in_=wqk_d[0])

        # packed constants (rope tables et al): needed once head-0's first
        # projection chain completes
        cpack = singles.tile([128, CP], BF)
        nc.sync.dma_start(out=cpack, in_=cpack_d[:])
        crep = cpack[:, 0:T]
        srep = cpack[:, T : 2 * T]
        ident = cpack[:, 2 * T : 2 * T + 128]
        mbias = cpack[:, 2 * T + 128 : 2 * T + 256]
        selc = cpack[0 : 2 * HL, 2 * T + 256 : 2 * T + 256 + HL * 128]

        for j in range(1, TC):
            nc.sync.dma_start(out=xt[:, j], in_=xt_d[:, j])

        wv_pool = ctx.enter_context(tc.tile_pool(name="wvpool", bufs=1))
        wv = wv_pool.tile([128, KT, CL], BF)
        nc.sync.dma_start(out=wv, in_=wv_d[:])
        wo_sb = [
            wo_pool.tile([128, D], BF, tag=f"wo{h}", name=f"wo{h}") for h in range(HL)
        ]

        # ---- helpers ----
        gcomb = singles.tile([2 * HL, T], BF)  # rows 0:4 g/8, rows 4:8 (1-g)/8

        def gate_chunk(j):
            tsl = slice(512 * j, 512 * (j + 1))
            pg = ps_s.tile([2 * HL, 512], F32, tag="ps", name="pg")
            for k in range(KT):
                nc.tensor.matmul(
                    pg, wg[:, k, :], xt[:, j, k, :], start=(k == 0), stop=(k == KT - 1)
                )
            gsig = sc_pool.tile([2 * HL, 512], F32, tag="gsig", bufs=1)
            nc.scalar.activation(
                gsig, pg, mybir.ActivationFunctionType.Sigmoid, bias=glog
            )
            nc.scalar.activation(
                gcomb[:, tsl],
                gsig,
                mybir.ActivationFunctionType.Identity,
                scale=gsv,
                bias=gbv,
            )

        def proj_chunk(h, j, wqk_sb, qstk, kstk):
            wsem_sb, wgeo_sb = wqk_sb[:, 0], wqk_sb[:, 1]
            """QK projection for head h, token chunk j.

            p_sem rows: [q_sem(0:64) | k_sem(64:128)]
            p_geo rows: [k_geo(0:64) | q_geo(64:128)]
            gbs  rows: [g/8   (0:64) | (1-g)/8 (64:128)]  (q-side scales)
            """
            tsl = slice(512 * j, 512 * (j + 1))
            p_sem = ps_big.tile([128, 512], F32, tag="big", name="p_sem")
            p_geo = ps_big.tile([128, 512], F32, tag="big", name="p_geo")
            for k in range(KT):
                nc.tensor.matmul(
                    p_sem, wsem_sb[:, k, :], xt[:, j, k, :],
                    start=(k == 0), stop=(k == KT - 1),
                )
            for k in range(KT):
                nc.tensor.matmul(
                    p_geo, wgeo_sb[:, k, :], xt[:, j, k, :],
                    start=(k == 0), stop=(k == KT - 1),
                )
            gbb = ps_big.tile([128, 512], F32, tag="big", name="gbb")
            nc.tensor.matmul(
                gbb, selc[:, 128 * h : 128 * (h + 1)], gcomb[:, tsl],
                start=True, stop=True,
            )
            gbs = sc_pool.tile([128, 512], BF, tag="gbs", bufs=2)
            nc.scalar.copy(gbs, gbb)

            # RoPE on the stacked geo tile (all 128 partitions per op)
            m1 = sc_pool.tile([128, 512], BF, tag="m1", bufs=2)
            m2 = sc_pool.tile([128, 512], BF, tag="m2", bufs=2)
            sw = sc_pool.tile([128, 512], BF, tag="sw", bufs=2)
            nc.vector.tensor_mul(m1, p_geo, crep[:, tsl])
            nc.vector.tensor_mul(m2, p_geo, srep[:, tsl])
            for blk in range(4):  # swap 32-row halves within each 64
                d0 = 64 * (blk // 2) + 32 * (blk % 2)
                s0 = 64 * (blk // 2) + 32 * (1 - blk % 2)
                # 2/2 DVE/ACT split measures best: all-DVE recreates vector
                # head-of-line stalls on the acc adds, all-ACT delays exps
                if blk % 2 == 0:
                    nc.vector.tensor_copy(sw[d0 : d0 + 32, :], m2[s0 : s0 + 32, :])
                else:
                    nc.scalar.copy(sw[d0 : d0 + 32, :], m2[s0 : s0 + 32, :])
            nc.vector.tensor_add(m1, m1, sw)  # m1 = rotated [k_geo | q_geo]

            # q side gets the gate scales folded in; k side is passthrough
            nc.vector.tensor_mul(qstk[0:64, tsl], p_sem[0:64, :], gbs[0:64, :])
            nc.vector.tensor_mul(qstk[64:128, tsl], m1[64:128, :], gbs[64:128, :])
            nc.vector.tensor_copy(kstk[0:64, tsl], p_sem[64:128, :])
            nc.vector.tensor_copy(kstk[64:128, tsl], m1[0:64, :])

        # ---- gate + head-0 projection, chunk by chunk (tracks DMA arrival) ----
        # ---- V projection helper, natural (t, dv) layout ----
        v_sb = vpool.tile([128, TT, CL], BF)

        def v_tile(i):
            pv = ps_big.tile([128, CL], F32, tag="big", name="pv")
            for k in range(KT):
                nc.tensor.matmul(
                    pv,
                    xt[:, i // 4, k, 128 * (i % 4) : 128 * (i % 4 + 1)],
                    wv[:, k, :],
                    start=(k == 0),
                    stop=(k == KT - 1),
                )
            if i % 2 == 0:
                nc.scalar.copy(v_sb[:, i, :], pv)
            else:
                nc.vector.tensor_copy(v_sb[:, i, :], pv)

        qstk0 = qk_pool.tile([128, T], BF, tag="qstk", name="qstk0")
        kstk0 = qk_pool.tile([128, T], BF, tag="kstk", name="kstk0")
        for j in range(TC):
            gate_chunk(j)
            proj_chunk(0, j, wqk0, qstk0, kstk0)
        for i in range(TT):
            v_tile(i)

        # ---- per-head: attention, then next head's projection ----
        outT = [
            ot_pool.tile([128, T], BF, tag=f"ot{h}", name=f"ot{h}") for h in range(HL)
        ]
        qstk, kstk = qstk0, kstk0
        for h in range(HL):
            for j in range(TC):
                tsl = slice(512 * j, 512 * (j + 1))
                po = ps_o.tile([128, 512], F32, tag="po")
                acc = sc_pool.tile([128, 512], BF, tag="acc", bufs=2)
                n_s = 4 * (j + 1)
                fold = []
                for s in range(n_s):
                    dj = s - 4 * j  # >=0 on diagonal tiles
                    c0 = 128 * dj if dj >= 0 else 0
                    ssl = slice(128 * s, 128 * (s + 1))
                    ps = ps_s.tile([128, 512], F32, tag="ps", name="ps")
                    nc.tensor.matmul(
                        ps[:, c0:512],
                        kstk[:, ssl],
                        qstk[:, 512 * j + c0 : 512 * (j + 1)],
                        start=True,
                        stop=(dj < 0),
                        skip_group_check=(dj >= 0),
                    )
                    if dj >= 0:
                        # causal mask: add -1e9 upper triangle to the diag block
                        nc.tensor.matmul(
                            ps[:, c0 : c0 + 128],
                            ident,
                            mbias,
                            start=False,
                            stop=True,
                            skip_group_check=True,
                        )
                    pt = p_pool.tile([128, 512], BF, tag="pt", name="pt")
                    nc.scalar.activation(
                        pt[:, c0:512], ps[:, c0:512], mybir.ActivationFunctionType.Exp
                    )
                    if s == 0:
                        nc.vector.tensor_copy(acc, pt)
                    elif dj < 3:
                        nc.vector.tensor_add(
                            acc[:, c0:512], acc[:, c0:512], pt[:, c0:512]
                        )
                    else:
                        fold.append((pt, c0))
                    nc.tensor.matmul(
                        po[:, c0:512],
                        v_sb[:, s, 128 * h : 128 * (h + 1)],
                        pt[:, c0:512],
                        start=(s == 0),
                        stop=(s == n_s - 1),
                    )
                # denominator: broadcast partition-sums of acc; the partial
                # diag tiles skip the DVE add chain and fold in as extra PE
                # accumulations so the chain never tails into the next chunk
                rbc = ps_r.tile([128, 512], F32, tag="rbc", name="rbc")
                nc.tensor.matmul(
                    rbc, ones128, acc, start=True, stop=False, skip_group_check=True
                )
                for fi, (fpt, fc0) in enumerate(fold):
                    nc.tensor.matmul(
                        rbc[:, fc0:512],
                        ones128,
                        fpt[:, fc0:512],
                        start=False,
                        stop=(fi == len(fold) - 1),
                        skip_group_check=True,
                    )
                rbs = sc_pool.tile([128, 512], F32, tag="rbs", bufs=2)
                nc.vector.reciprocal_approx_fast(out=rbs, in_=rbc)
                nc.vector.tensor_mul(outT[h][:, tsl], po, rbs)

            if h + 1 < HL:
                wqk_sb = wqk_pool.tile([128, 2, KT, 128], BF, tag="wqk")
                nc.sync.dma_start(out=wqk_sb, in_=wqk_d[h + 1])
                if h == 2:  # out-proj weights: loads due ~250us, issue late
                    for hh in range(HL):
                        nc.sync.dma_start(out=wo_sb[hh], in_=wo_d[hh])
                qstk = qk_pool.tile([128, T], BF, tag="qstk")
                kstk = qk_pool.tile([128, T], BF, tag="kstk")
                for j in range(TC):
                    proj_chunk(h + 1, j, wqk_sb, qstk, kstk)

        # ---- out-projection: y[t, e] = sum_h outT_h^T @ wo_h ----
        for i in range(TT):
            ysb = y_pool.tile([128, D], BF, tag="ysb")
            for ec in range(D // 512):
                py = ps_big.tile([128, 512], F32, tag="big", name="py")
                for h in range(HL):
                    nc.tensor.matmul(
                        py,
                        outT[h][:, 128 * i : 128 * (i + 1)],
                        wo_sb[h][:, 512 * ec : 512 * (ec + 1)],
                        start=(h == 0),
                        stop=(h == HL - 1),
                    )
                if ec % 2 == 0:
                    nc.scalar.copy(ysb[:, 512 * ec : 512 * (ec + 1)], py)
                else:
                    nc.vector.tensor_copy(ysb[:, 512 * ec : 512 * (ec + 1)], py)
                if i >= TT - 2:
                    # stream the final tiles' stores so the end-of-kernel
                    # barrier isn't gated on one large trailing transfer
                    nc.scalar.dma_start(
                        out=y_d[128 * i : 128 * (i + 1), 512 * ec : 512 * (ec + 1)],
                        in_=ysb[:, 512 * ec : 512 * (ec + 1)],
                    )
            if i < TT - 2:
                nc.scalar.dma_start(out=y_d[128 * i : 128 * (i + 1), :], in_=ysb)

    nc.finalize()
    return nc


def _host_prep(x, w_q_sem, w_k_sem, w_q_geo, w_k_geo, w_v, w_out, gate_logit, gate_w):
    """Build the 8 per-core input maps (all numpy, bf16 where matmul-bound)."""
    half = GEO_HD // 2  # 32
    inv_freq = 1.0 / (ROPE_BASE ** (np.arange(half, dtype=np.float64) / half))
    pos = np.arange(T, dtype=np.float64)
    ang = pos[None, :] * inv_freq[:, None]  # (32, T)
    cos, sin = np.cos(ang), np.sin(ang)
    crep = np.empty((128, T), dtype=NPBF)
    srep = np.empty((128, T), dtype=NPBF)
    for b0 in (0, 64):
        crep[b0 : b0 + 32] = cos
        crep[b0 + 32 : b0 + 64] = cos
        srep[b0 : b0 + 32] = sin  # sw[0:32]=m2[32:64] needs +sin here
        srep[b0 + 32 : b0 + 64] = -sin  # sw[32:64]=m2[0:32] needs -sin here
    # rot[0:32] = p[0:32]*cos - p[32:64]*sin = m1[0:32] + (p[32:64]*srep[32:64])
    # rot[32:64] = p[32:64]*cos + p[0:32]*sin = m1[32:64] + (p[0:32]*srep[0:32])
    # (sw swaps the 32-blocks, so srep rows carry the sign of the *destination*)

    p_i = np.arange(128)
    mbias = np.where(p_i[:, None] <= p_i[None, :], 0.0, NEG_INF).astype(NPBF)
    ident = np.eye(128, dtype=NPBF)
    selc = np.zeros((128, HL * 128), dtype=NPBF)
    for h in range(HL):
        selc[h, 128 * h : 128 * h + 64] = 1.0
        selc[HL + h, 128 * h + 64 : 128 * h + 128] = 1.0
    cpack = np.concatenate(
        [crep, srep, ident, mbias, selc], axis=1
    )  # (128, 2T+256+512)

    def stack_heads(wa, wb):
        # per-head (D, 128) = [wa_head | wb_head], as (128, KT, 128) lhsT tiles
        out = []
        for h in range(H):
            blk = np.concatenate(
                [wa[:, 64 * h : 64 * (h + 1)], wb[:, 64 * h : 64 * (h + 1)]], axis=1
            )
            out.append(
                np.ascontiguousarray(
                    blk.reshape(KT, 128, 128).transpose(1, 0, 2)
                ).astype(NPBF)
            )
        return out

    wsem_all = stack_heads(w_q_sem, w_k_sem)  # [q_sem | k_sem]
    wgeo_all = stack_heads(w_k_geo, w_q_geo)  # [k_geo | q_geo]
    wqk_all = [
        np.ascontiguousarray(np.stack([ws, wgg], axis=1))  # (128, 2, KT, 128)
        for ws, wgg in zip(wsem_all, wgeo_all)
    ]

    xt_by_b = [
        np.ascontiguousarray(
            x[b].T.reshape(KT, 128, TC, 512).transpose(1, 2, 0, 3)
        ).astype(NPBF)
        for b in range(B)
    ]  # (128, TC, KT, 512): [p, j, k, c] = xT[128k+p, 512j+c]

    in_maps = []
    for core in range(8):
        b, hg = core // 4, core % 4
        heads = range(4 * hg, 4 * hg + 4)
        wqk = np.stack([wqk_all[h] for h in heads])
        wv = np.ascontiguousarray(
            w_v[:, CL * hg : CL * (hg + 1)].reshape(KT, 128, CL).transpose(1, 0, 2)
        ).astype(NPBF)
        wo = w_out[CL * hg : CL * (hg + 1), :].reshape(HL, 128, D).astype(NPBF)
        gwl = gate_w[:, 4 * hg : 4 * hg + 4]  # (D, 4)
        gw2 = np.concatenate([gwl, gwl], axis=1)  # (D, 8) duplicated
        wg = np.ascontiguousarray(
            gw2.reshape(KT, 128, 2 * HL).transpose(1, 0, 2)
        ).astype(NPBF)
        gll = gate_logit[4 * hg : 4 * hg + 4]
        glog = np.concatenate([gll, gll]).astype(np.float32)
        gsv = np.array([0.125] * HL + [-0.125] * HL, dtype=np.float32)
        gbv = np.array([0.0] * HL + [0.125] * HL, dtype=np.float32)
        gpack = np.ascontiguousarray(np.stack([glog, gsv, gbv], axis=1))  # (8, 3)
        in_maps.append(
            {
                "xt": xt_by_b[b],
                "wqk": wqk,
                "wv": wv,
                "wo": np.ascontiguousarray(wo),
                "wg": wg,
                "cpack": cpack,
                "gpack": gpack,
            }
        )
    return in_maps


def _run(inputs, trace=False):
    global _CACHED_NC
    if _CACHED_NC is None:
        _CACHED_NC = _build_nc()
    in_maps = _host_prep(**{k: np.asarray(v) for k, v in inputs.items()})
    res = run_bass_kernel_spmd(
        _CACHED_NC, in_maps, core_ids=list(range(8)), trace=trace
    )
    y = np.zeros((B, T, D), dtype=np.float32)
    for core in range(8):
        y[core // 4] += res.results[core]["y"].astype(np.float32)
    return y, res


def kernel(**inputs) -> np.ndarray:
    y, _ = _run(inputs, trace=False)
    return y
